# revision 28
# baseline (speedup 1.0000x reference)
"""Multi-head self-attention (B=4, T=2048, E=1024, H=16, Dh=64) on 8 trn2 cores.

Sharding (tensor-parallel over heads + data-parallel over batch, per the
problem's sharding hint): core c handles batch b=c//2 and head-half s=c%2
(8 of 16 heads), with ALL 2048 queries of its batch. Each core computes
q/k/v projections for its 512 head-dims, attention for its 8 heads, and a
PARTIAL output projection (contracting only its 512 rows of fc_w). The two
partials per batch are summed on the host (the TP all-reduce), plus fc_b.

All matmul operands are fp16 (fp32 PSUM accumulation): fp16 streams 1
col/cycle through the PE vs 0.5 for fp32. Softmax denominators come free
from a ones-column in the packed V block; reciprocals are taken on the two
denominator rows BEFORE the mask-matmul broadcast. exp() runs as [128,2048]
ACTIVATEs (4 PSUM banks) to amortize the ~350cyc fixed cost.

Schedule: xt tiles are DMA'd ONCE and stay resident for both head groups;
all weight DMAs (both groups + fc) are prefetched at kernel start, split
across the two HWDGE queues (sync + scalar engines) so the first
projection chain starts as early as possible. Phase A (projections) of
group 1 feeds into phase B (attention) of group 0, and fc matmuls feed
into phase B of group 1, so the PE queue always has fill work during exp
stalls. Six group-1 q-projection chunks are held back to feed the first
B(1) iterations, and window 3's fc is split into two half-contractions so
the drain after the last attention iteration is minimal. Output is stored
fp16 (partials; exact sum happens on host in fp32).

Per-core math (heads processed in NG=2 groups of 4; head pair p packs its
even head at partitions 0:64 and odd head at 64:128):
  xt        [E, T]            (input, host pre-transposed/tiled, fp16)
  qT_j      [64, 2048] = Wq_j.T @ xt + bq_j      (slab-packed)
  kT_j      [64, 2048] = Wk_j.T @ xt             (k-bias is softmax-invariant)
  v_j       [2048, 64] = xt.T @ Wv_j + bv_j      (+ ones column for denom)
  e         [k,q] blocks via K=64 matmul over the slabs
  p = exp(e/8);  psO = [v | ones].T @ p  -> AV rows + denominator row
  outT      [64, q] per head = AV rows * recip(denom) (broadcast via dmask)
  out_part  [2048, 1024] = outT.T @ fc_w[s*512:(s+1)*512, :]   (partial)
"""

import numpy as np

E = 1024
T = 2048          # tokens per batch (= queries = keys per core)
H = 16
DH = 64
EC = E // 128     # 8 e-chunks
NG = 2            # head groups per core
HPG = 4           # heads per group
GC = HPG // 2     # head pairs per group = 2
NH = NG * HPG     # heads per core = 8
HD = NH * DH      # head-dims per core = 512
N_CORES = 8
SCALE = DH ** -0.5
KC = T // 128     # 16 key chunks
NTB = T // 512    # 4 token blocks
NQB = T // 512    # 4 query windows

_CACHE = {}


def _build():
    import concourse.bass as bass
    import concourse.mybir as mybir
    import concourse.tile as tile
    from concourse import bacc
    from contextlib import ExitStack

    f32 = mybir.dt.float32
    f16 = mybir.dt.float16
    AF = mybir.ActivationFunctionType
    OP = mybir.AluOpType

    nc = bacc.Bacc("TRN2", target_bir_lowering=False, debug=False)

    xt4 = nc.declare_dram_parameter("xt4", [NTB, 128, EC * 512], f16, isOutput=False).ap()
    wqkv = nc.declare_dram_parameter("wqkv", [NG, 3, 128, EC * 256], f16, isOutput=False).ap()
    q_bias = nc.declare_dram_parameter("q_bias", [128, NG * GC, 1], f32, isOutput=False).ap()
    v_bias = nc.declare_dram_parameter("v_bias", [128, HD], f32, isOutput=False).ap()
    fc_w = nc.declare_dram_parameter("fc_w", [128, (HD // 128) * E], f16, isOutput=False).ap()
    out = nc.declare_dram_parameter("out", [T, E], f16, isOutput=True).ap()

    with tile.TileContext(nc) as tc, ExitStack() as ctx:
        pool_const = ctx.enter_context(tc.tile_pool(name="const", bufs=1))
        pool_kqv = ctx.enter_context(tc.tile_pool(name="kqv", bufs=1))
        pool_w = ctx.enter_context(tc.tile_pool(name="w", bufs=2))
        pool_xt = ctx.enter_context(tc.tile_pool(name="xt", bufs=1))
        pool_ex = ctx.enter_context(tc.tile_pool(name="ex", bufs=4))
        pool_outT = ctx.enter_context(tc.tile_pool(name="outT", bufs=1))
        pool_fc = ctx.enter_context(tc.tile_pool(name="fc", bufs=1))
        pool_ot = ctx.enter_context(tc.tile_pool(name="ot", bufs=4))
        ps_mm = ctx.enter_context(tc.tile_pool(name="psmm", bufs=2, space="PSUM"))
        ps_e = ctx.enter_context(tc.tile_pool(name="pse", bufs=1, space="PSUM"))
        ps_o = ctx.enter_context(tc.tile_pool(name="pso", bufs=1, space="PSUM"))

        # ---- persistent tiles ----
        vb_sb = pool_const.tile([128, HD], f32)
        qb_sb = pool_const.tile([128, NG * GC, 1], f32)
        fcw_sb = pool_fc.tile([128, HD // 128, E], f16)

        xts = [pool_xt.tile([128, EC, 512], f16, name=f"xt{tb}", tag=f"xt{tb}")
               for tb in range(NTB)]
        wks = [pool_w.tile([128, EC, 256], f16, name=f"wk{g}", tag="wk")
               for g in range(NG)]
        # both groups' v-weights side by side: one N=512 vproj matmul covers
        # all 8 heads' v-dims, halving vproj instruction (and LdWeights) count
        wv_all = pool_w.tile([128, EC, 2 * 256], f16, name="wv_all", tag="wv",
                             bufs=1)
        wqs = [pool_w.tile([128, EC, 256], f16, name=f"wq{g}", tag="wq")
               for g in range(NG)]

        # ---- startup DMAs: interleave across the two HWDGE queues so the
        # first kproj chain (wk + xt0) lands as early as possible; everything
        # else (incl. group-1 weights and fc weights) streams in behind.
        def src_w(g, m):
            return wqkv[g, m].rearrange("p (c n) -> p c n", c=EC)

        def src_xt(tb):
            return xt4[tb].rearrange("p (c n) -> p c n", c=EC)

        # first working set (wk0 + xt0) in ec-pair chunks, interleaved across
        # the two queues, so the first kproj chain starts ~1us after the
        # first pair lands instead of waiting for whole tiles
        wk_src = src_w(0, 1)
        xt0_src = src_xt(0)
        for h in range(4):
            sl = slice(2 * h, 2 * h + 2)
            nc.sync.dma_start(out=wks[0][:, sl, :], in_=wk_src[:, sl, :])
            nc.scalar.dma_start(out=xts[0][:, sl, :], in_=xt0_src[:, sl, :])
        nc.sync.dma_start(out=wv_all[:, :, 0:256], in_=src_w(0, 2))
        nc.sync.dma_start(out=wv_all[:, :, 256:512], in_=src_w(1, 2))
        nc.scalar.dma_start(out=xts[1], in_=src_xt(1))
        nc.sync.dma_start(out=vb_sb, in_=v_bias)
        nc.sync.dma_start(out=qb_sb, in_=q_bias)
        nc.sync.dma_start(out=wqs[0], in_=src_w(0, 0))

        # denominator-broadcast mask: psR = dmask.T @ dsb maps dsb row 64
        # (even-head denom) onto rows 0:64 and dsb row 0 (odd) onto 64:128
        dmask = pool_const.tile([128, 128], f16)
        nc.vector.memset(dmask, 0.0)
        nc.vector.memset(dmask[64:65, 0:64], 1.0)
        nc.vector.memset(dmask[0:1, 64:128], 1.0)

        # warm the exp table (the implicit ACT_TABLE_LOAD precedes this on
        # the scalar queue, after the xt0/xt1 triggers above)
        dummy = pool_const.tile([128, 1], f16)
        nc.scalar.activation(dummy, dmask[:, 0:1], AF.Exp, scale=0.125)

        nc.scalar.dma_start(out=xts[2], in_=src_xt(2))
        nc.scalar.dma_start(out=xts[3], in_=src_xt(3))
        nc.sync.dma_start(out=wks[1], in_=src_w(1, 1))
        nc.sync.dma_start(out=wqs[1], in_=src_w(1, 0))
        nc.sync.dma_start(out=fcw_sb,
                          in_=fc_w.rearrange("p (c n) -> p c n", c=HD // 128))

        dsbs = []
        for i in range(2):
            d = pool_const.tile([128, 512], f16, name=f"dsb{i}")
            nc.vector.memset(d, 0.0)
            dsbs.append(d)

        # p-state warmup: the PE ramps 0.65 -> 2.4 GHz over ~3us of
        # continuous work. Burn junk matmuls (never read) while the first
        # weight/xt DMAs are in flight so the real chains start at full
        # clock. These have no data deps beyond the dmask/dsb memsets.
        for i in range(6):
            psJ = ps_mm.tile([128, 512], f32, tag="mm", name=f"psJ{i}")
            nc.tensor.matmul(psJ, dmask, dsbs[0], start=True, stop=True)

        # double-buffered k/q/v group tiles. Only va needs its ones/zero
        # strips initialized; the kT/qT off-halves are never read (QK
        # contracts over just the 64 data rows of each slab).
        kTs, qTs, vas = [], [], []
        for i in range(2):
            kT = pool_kqv.tile([128, HPG, T], f16, name=f"kT{i}", tag=f"kT{i}")
            qT = pool_kqv.tile([128, HPG, T], f16, name=f"qT{i}", tag=f"qT{i}")
            va = pool_kqv.tile([128, KC, GC, 192], f16, name=f"va{i}", tag=f"va{i}")
            nc.gpsimd.memset(va[:, :, :, 64:65], 1.0)
            nc.gpsimd.memset(va[:, :, :, 65:128], 0.0)
            kTs.append(kT)
            qTs.append(qT)
            vas.append(va)

        outT = pool_outT.tile([128, HD // 128, T], f16)

        # ---- v-projection: one unit covers BOTH groups (wv_all moving
        # operand is 512 wide), so it lives outside a_chunks.
        def mk_vproj(tb, tc_):
            def f():
                xt_tb = xts[tb]
                psV = ps_mm.tile([128, 512], f32, tag="mm")
                for ec in range(EC):
                    nc.tensor.matmul(
                        psV, xt_tb[:, ec, tc_ * 128:(tc_ + 1) * 128],
                        wv_all[:, ec, :],
                        start=(ec == 0), stop=(ec == EC - 1))
                K0 = tb * 4 + tc_
                for g in range(NG):
                    psv4 = psV[:, g * 256:(g + 1) * 256].rearrange(
                        "p (pr h d) -> p pr h d", pr=GC, h=2)
                    vb4 = vb_sb[:, g * 256:(g + 1) * 256].rearrange(
                        "p (pr h d) -> p pr h d", pr=GC, h=2)
                    va = vas[g]
                    nc.vector.tensor_add(
                        va[:, K0, :, 0:64], psv4[:, :, 0, :], vb4[:, :, 0, :])
                    nc.vector.tensor_add(
                        va[:, K0, :, 128:192], psv4[:, :, 1, :], vb4[:, :, 1, :])
            return f

        # ---- phase A: k/q projections for group g, as chunk closures.
        # Returns (pre, deferred, held):
        #   pre      — needed by b_iter(g, 0, 0): k-proj pair 0 and q-proj
        #              window 0 pair 0
        #   deferred — needed by b_iter(g, 0, 1): k-proj pair 1, q window 0
        #              pair 1 (fed during the first b_iter)
        #   held     — q-projs for windows 1-3 (fed ahead of their window)
        def a_chunks(g):
            gi = g % 2
            kT, qT = kTs[gi], qTs[gi]
            wq, wk = wqs[g], wks[g]

            def mk_kproj(tb, ch):
                def f():
                    xt_tb = xts[tb]
                    psA = ps_mm.tile([128, 512], f32, tag="mm")
                    for ec in range(EC):
                        nc.tensor.matmul(
                            psA, wk[:, ec, ch * 128:(ch + 1) * 128], xt_tb[:, ec, :],
                            start=(ec == 0), stop=(ec == EC - 1))
                    nc.vector.tensor_copy(
                        kT[0:64, 2 * ch, tb * 512:(tb + 1) * 512], psA[0:64, :])
                    nc.vector.tensor_copy(
                        kT[64:128, 2 * ch + 1, tb * 512:(tb + 1) * 512], psA[64:128, :])
                return f

            def mk_qproj(tb, ch):
                def f():
                    xt_tb = xts[tb]
                    P = g * GC + ch
                    psQ = ps_mm.tile([128, 512], f32, tag="mm")
                    for ec in range(EC):
                        nc.tensor.matmul(
                            psQ, wq[:, ec, ch * 128:(ch + 1) * 128], xt_tb[:, ec, :],
                            start=(ec == 0), stop=(ec == EC - 1))
                    nc.vector.tensor_scalar(
                        qT[0:64, 2 * ch, tb * 512:(tb + 1) * 512],
                        psQ[0:64, :], qb_sb[0:64, P, :], None, OP.add)
                    nc.vector.tensor_scalar(
                        qT[64:128, 2 * ch + 1, tb * 512:(tb + 1) * 512],
                        psQ[64:128, :], qb_sb[64:128, P, :], None, OP.add)
                return f

            pre, deferred, held = [], [], []
            for tb in range(NTB):
                pre.append(mk_kproj(tb, 0))
                deferred.append(mk_kproj(tb, 1))
            pre.append(mk_qproj(0, 0))
            deferred.append(mk_qproj(0, 1))
            for tb in range(1, NTB):
                held.append(mk_qproj(tb, 0))
                held.append(mk_qproj(tb, 1))
            return pre, deferred, held

        # ---- phase C: fc for one 128-query block (both 512-col halves).
        # hcs selects the contraction slices (head-pair blocks of fc rows).
        def fc_mm(qc, ob, hcs, psC, start):
            for i, hc in enumerate(hcs):
                nc.tensor.matmul(
                    psC, outT[:, hc, qc * 128:(qc + 1) * 128],
                    fcw_sb[:, hc, ob * 512:(ob + 1) * 512],
                    start=(start and i == 0), stop=(i == len(hcs) - 1))

        # fc feed granularity is a half-block (one 512-col half of one
        # 128-query block): ~2k PE cycles per feed item. The output DMA goes
        # out with the second half.
        ots = {}

        def fc_half(qc, ob):
            if ob == 0:
                ots[qc] = pool_ot.tile([128, E], f16, name=f"ot{qc}", tag="ot")
            ot = ots[qc]
            psC = ps_mm.tile([128, 512], f32, tag="mm")
            fc_mm(qc, ob, (0, 1, 2, 3), psC, True)
            nc.vector.tensor_copy(ot[:, ob * 512:(ob + 1) * 512], psC)
            if ob == 1:
                nc.sync.dma_start(out=out[qc * 128:(qc + 1) * 128, :], in_=ot)

        # split fc for the last window: partA (group-0 head pairs, ready
        # right after B(0)) runs early in B(1); partB finishes after the
        # last attention iteration with only 2 matmuls per half left.
        accs = {}

        def fc_partA(qc, ob):
            if ob == 0:
                accs[qc] = pool_ot.tile([128, E], f16, name=f"acc{qc}",
                                        tag=f"acc{qc % 4}", bufs=1)
            acc = accs[qc]
            psC = ps_mm.tile([128, 512], f32, tag="mm")
            fc_mm(qc, ob, (0, 1), psC, True)
            nc.vector.tensor_copy(acc[:, ob * 512:(ob + 1) * 512], psC)

        def fc_partB(qc, ob):
            acc = accs[qc]
            if ob == 0:
                ots[qc] = pool_ot.tile([128, E], f16, name=f"otb{qc}", tag="ot")
            ot = ots[qc]
            psC = ps_mm.tile([128, 512], f32, tag="mm")
            fc_mm(qc, ob, (2, 3), psC, True)
            osl = slice(ob * 512, (ob + 1) * 512)
            if qc < 14:
                # DVE path: add straight out of PSUM
                nc.vector.tensor_add(ot[:, osl], psC, acc[:, osl])
            else:
                # ACT+gpsimd path (both idle at the tail): ACT copies PSUM ->
                # SBUF, gpsimd (SBUF-only) adds. Halves the tail drain chain.
                tmp = pool_ot.tile([128, 512], f16, name=f"tmpB{qc}_{ob}",
                                   tag=f"tmpB{ob}", bufs=1)
                nc.scalar.copy(tmp, psC)
                nc.gpsimd.tensor_add(ot[:, osl], tmp, acc[:, osl])
            if ob == 1:
                nc.sync.dma_start(out=out[qc * 128:(qc + 1) * 128, :], in_=ot)

        # ---- phase B: attention for (g, qb-window, head pair p) ----
        def b_iter(g, qb, p, feeder, it, feed_ks):
            gi = g % 2
            kT, qT, va = kTs[gi], qTs[gi], vas[gi]
            psO_e = ps_o.tile([128, 512], f32, tag="po_e")
            psO_o = ps_o.tile([128, 512], f32, tag="po_o")

            def av(K, ex):
                blk = va[:, K, p, :]
                # even head: 65 stationary cols (64 dims + ones) suffice —
                # psO_e rows 65:127 are never read
                nc.tensor.matmul(
                    psO_e[0:65, :], blk[:, 0:65], ex[:, 0, :],
                    start=(K == 0), stop=(K == KC - 1))
                nc.tensor.matmul(
                    psO_o, blk[:, 64:192], ex[:, 1, :],
                    start=(K == 0), stop=(K == KC - 1))

            # software pipeline: AV of chunk K-1 is emitted after QK/ACT of
            # chunk K, so the in-order PE queue never stalls on ACT(K)
            prev = None
            for K in range(KC):
                psE = ps_e.tile([128, 2, 512], f32, tag=f"pe{K % 2}")
                for hf in range(2):
                    j = p * 2 + hf
                    # contract over just the 64 data rows of the slab
                    # (K=64): halves the stationary load vs the padded K=128
                    lo = 0 if j % 2 == 0 else 64
                    nc.tensor.matmul(
                        psE[:, hf, :],
                        kT[lo:lo + 64, j, K * 128:(K + 1) * 128],
                        qT[lo:lo + 64, j, qb * 512:(qb + 1) * 512],
                        start=True, stop=True)
                ex = pool_ex.tile([128, 2, 512], f16, tag="ex")
                nc.scalar.activation(ex, psE, AF.Exp, scale=SCALE)
                if prev is not None:
                    av(*prev)
                if K in feed_ks:
                    feeder()  # fill remaining slack with proj/fc matmuls
                prev = (K, ex)
            av(*prev)
            dsb = dsbs[it % 2]
            nc.vector.tensor_copy(dsb[64:65, :], psO_e[64:65, :])
            nc.vector.tensor_copy(dsb[0:1, :], psO_o[0:1, :])
            psR = ps_mm.tile([128, 512], f32, tag="mm")
            # rows 1..63 of dmask/dsb are all-zero: K=65 covers both
            # nonzero rows (0 and 64) exactly
            nc.tensor.matmul(psR, dmask[0:65, :], dsb[0:65, :],
                             start=True, stop=True)
            recipb = pool_ex.tile([128, 512], f32, tag="recipb")
            nc.vector.reciprocal_approx_fast(out=recipb, in_=psR)
            P = g * GC + p
            qs = slice(qb * 512, (qb + 1) * 512)
            nc.vector.tensor_tensor(
                outT[0:64, P, qs], psO_e[0:64, :], recipb[0:64, :], OP.mult)
            nc.vector.tensor_tensor(
                outT[64:128, P, qs], psO_o[64:128, :], recipb[64:128, :], OP.mult)

        # ---- main schedule ----
        # Only the chunks b_iter(0,0,0) needs run before B(0); everything
        # else is fill work fed into attention iterations, so the PE never
        # idles while the ACT engine streams exps.
        a0_pre, a0_def, a0_held = a_chunks(0)
        a1_pre, a1_def, a1_held = a_chunks(1)
        # A(0) serial part: group-0 k-pair0 + ALL v (both groups, merged
        # units) + q window 0 pair 0
        for tb in range(NTB):
            a0_pre[tb]()                      # kproj(tb, 0)
            for tc_ in range(4):
                mk_vproj(tb, tc_)()
        a0_pre[-1]()                          # qproj(0, 0)

        # B(0) feed queue: pair-1 projections of group 0 first (needed by
        # iter 1), then group-0 held q-projs interleaved with group-1
        # k-chunks (held q-proj (tb,*) must land before window tb's iters).
        a1_k = a1_pre[:-1]       # 4 chunks: k-pair0
        # q(0,1) right after the first deferred k chunk so iter 1's QK(0)
        # unblocks early; remaining deferred k chunks land just ahead of the
        # key chunks that read them
        feed_items = [a0_def[0], a0_def[4], a0_def[1], a0_def[2], a0_def[3]]
        feed_items += a0_held[0:2] + a1_k[0:2]
        feed_items += a0_held[2:4] + a1_k[2:4]
        feed_items += a0_held[4:6]
        feed_items += a1_def + [a1_pre[-1]]   # k-pair1, q(0,1), q(0,0)
        state = {"i": 0}

        def feeder():
            if state["i"] < len(feed_items):
                feed_items[state["i"]]()
                state["i"] += 1

        it = 0
        for qb in range(NQB):
            for p in range(GC):
                b_iter(0, qb, p, feeder, it, feed_ks=(2, 7, 12))
                it += 1
        while state["i"] < len(feed_items):
            feeder()

        # B(1) feed queue: held q-projs ahead of their windows, window-3 fc
        # partA (group-0 contraction half) early, fc halves as windows
        # complete, partB after the last iteration.
        feed_items.extend(a1_held[0:2])
        for qc in (12, 13):
            feed_items.append(lambda qc=qc: fc_partA(qc, 0))
            feed_items.append(lambda qc=qc: fc_partA(qc, 1))
        feed_items.extend(a1_held[2:4])
        for qc in (14, 15):
            feed_items.append(lambda qc=qc: fc_partA(qc, 0))
            feed_items.append(lambda qc=qc: fc_partA(qc, 1))
        feed_items.extend(a1_held[4:6])
        for qb in range(NQB):
            for p in range(GC):
                b_iter(1, qb, p, feeder, it, feed_ks=(1, 4, 7, 10, 13))
                it += 1
            if qb < 3:
                for qc in range(qb * 4, (qb + 1) * 4):
                    for ob in range(2):
                        feed_items.append(lambda qc=qc, ob=ob: fc_half(qc, ob))
        while state["i"] < len(feed_items):
            feeder()
        for qc in range(12, 16):
            fc_partB(qc, 0)
            fc_partB(qc, 1)

    nc.compile()
    return nc


def _get_nc():
    if "nc" not in _CACHE:
        _CACHE["nc"] = _build()
    return _CACHE["nc"]


def _in_maps(x, qkv_w, qkv_b, fc_w, fc_b):
    f16 = np.float16
    x = np.asarray(x, dtype=np.float32)
    qkv_w = np.asarray(qkv_w, dtype=np.float32)
    qkv_b = np.asarray(qkv_b, dtype=np.float32)
    fc_w = np.asarray(fc_w, dtype=np.float32)

    maps = []
    for c in range(N_CORES):
        b, s = c // 2, c % 2
        # xt4[tb, p, ec*512 + t] = x[b, tb*512 + t, ec*128 + p]
        xt4 = np.ascontiguousarray(
            x[b].astype(f16).reshape(NTB, 512, EC, 128).transpose(0, 3, 2, 1)
            .reshape(NTB, 128, EC * 512))
        # wqkv[g, m, p, ec*256 + n] = W_m[ec*128 + p, s*512 + g*256 + n]
        wg = np.empty((NG, 3, 128, EC * 256), dtype=f16)
        for m in range(3):
            sub = qkv_w[:, m * E + s * HD:m * E + (s + 1) * HD]  # [E, 512]
            for g in range(NG):
                blk = sub[:, g * 256:(g + 1) * 256]              # [E, 256]
                wg[g, m] = (blk.reshape(EC, 128, 256).transpose(1, 0, 2)
                            .reshape(128, EC * 256).astype(f16))
        qb = np.ascontiguousarray(
            qkv_b[s * HD:(s + 1) * HD].reshape(NG * GC, 128).T.reshape(128, NG * GC, 1),
            dtype=np.float32)
        vb = np.ascontiguousarray(
            np.broadcast_to(qkv_b[2 * E + s * HD:2 * E + (s + 1) * HD], (128, HD)),
            dtype=np.float32)
        # fc_ws[p, hc*1024 + n] = fc_w[s*512 + hc*128 + p, n]
        fcs = np.ascontiguousarray(
            fc_w[s * HD:(s + 1) * HD, :].reshape(HD // 128, 128, E)
            .transpose(1, 0, 2).reshape(128, (HD // 128) * E).astype(f16))
        maps.append({"xt4": xt4, "wqkv": wg, "q_bias": qb, "v_bias": vb,
                     "fc_w": fcs})
    return maps


def run(x, qkv_w, qkv_b, fc_w, fc_b, trace=False):
    from concourse.bass_utils import run_bass_kernel_spmd

    nc = _get_nc()
    maps = _in_maps(x, qkv_w, qkv_b, fc_w, fc_b)
    res = run_bass_kernel_spmd(nc, maps, list(range(N_CORES)), trace=trace)
    B = np.asarray(x).shape[0]
    fc_b = np.asarray(fc_b, dtype=np.float32)
    full = np.empty((B, T, E), dtype=np.float32)
    for b in range(B):
        full[b] = (res.results[2 * b]["out"].astype(np.float32)
                   + res.results[2 * b + 1]["out"].astype(np.float32) + fc_b)
    return full, res


def kernel(x, qkv_w, qkv_b, fc_w, fc_b):
    full, _ = run(x, qkv_w, qkv_b, fc_w, fc_b, trace=False)
    return full


# revision 35
# speedup vs baseline: 1.0154x; 1.0154x over previous
"""Multi-head self-attention (B=4, T=2048, E=1024, H=16, Dh=64) on 8 trn2 cores.

Sharding (tensor-parallel over heads + data-parallel over batch, per the
problem's sharding hint): core c handles batch b=c//2 and head-half s=c%2
(8 of 16 heads), with ALL 2048 queries of its batch. Each core computes
q/k/v projections for its 512 head-dims, attention for its 8 heads, and a
PARTIAL output projection (contracting only its 512 rows of fc_w). The two
partials per batch are summed on the host (the TP all-reduce), plus fc_b.

All matmul operands are fp16 (fp32 PSUM accumulation): fp16 streams 1
col/cycle through the PE vs 0.5 for fp32. Softmax denominators come free
from a ones-column in the packed V block; reciprocals are taken on the two
denominator rows BEFORE the mask-matmul broadcast. exp() runs as [128,2048]
ACTIVATEs (4 PSUM banks) to amortize the ~350cyc fixed cost.

Schedule: xt tiles are DMA'd ONCE and stay resident for both head groups;
all weight DMAs (both groups + fc) are prefetched at kernel start, split
across the two HWDGE queues (sync + scalar engines) so the first
projection chain starts as early as possible. Phase A (projections) of
group 1 feeds into phase B (attention) of group 0, and fc matmuls feed
into phase B of group 1, so the PE queue always has fill work during exp
stalls. Six group-1 q-projection chunks are held back to feed the first
B(1) iterations, and window 3's fc is split into two half-contractions so
the drain after the last attention iteration is minimal. Output is stored
fp16 (partials; exact sum happens on host in fp32).

Per-core math (heads processed in NG=2 groups of 4; head pair p packs its
even head at partitions 0:64 and odd head at 64:128):
  xt        [E, T]            (input, host pre-transposed/tiled, fp16)
  qT_j      [64, 2048] = Wq_j.T @ xt + bq_j      (slab-packed)
  kT_j      [64, 2048] = Wk_j.T @ xt             (k-bias is softmax-invariant)
  v_j       [2048, 64] = xt.T @ Wv_j + bv_j      (+ ones column for denom)
  e         [k,q] blocks via K=64 matmul over the slabs
  p = exp(e/8);  psO = [v | ones].T @ p  -> AV rows + denominator row
  outT      [64, q] per head = AV rows * recip(denom) (broadcast via dmask)
  out_part  [2048, 1024] = outT.T @ fc_w[s*512:(s+1)*512, :]   (partial)
"""

import numpy as np

E = 1024
T = 2048          # tokens per batch (= queries = keys per core)
H = 16
DH = 64
EC = E // 128     # 8 e-chunks
NG = 2            # head groups per core
HPG = 4           # heads per group
GC = HPG // 2     # head pairs per group = 2
NH = NG * HPG     # heads per core = 8
HD = NH * DH      # head-dims per core = 512
N_CORES = 8
SCALE = DH ** -0.5
KC = T // 128     # 16 key chunks
NTB = T // 512    # 4 token blocks
NQB = T // 512    # 4 query windows

_CACHE = {}


def _build():
    import concourse.bass as bass
    import concourse.mybir as mybir
    import concourse.tile as tile
    from concourse import bacc
    from contextlib import ExitStack

    f32 = mybir.dt.float32
    f16 = mybir.dt.float16
    AF = mybir.ActivationFunctionType
    OP = mybir.AluOpType

    nc = bacc.Bacc("TRN2", target_bir_lowering=False, debug=False)

    xt4 = nc.declare_dram_parameter("xt4", [NTB, 128, EC * 512], f16, isOutput=False).ap()
    wqkv = nc.declare_dram_parameter("wqkv", [NG, 3, 128, EC * 256], f16, isOutput=False).ap()
    q_bias = nc.declare_dram_parameter("q_bias", [128, NG * GC, 1], f32, isOutput=False).ap()
    v_bias = nc.declare_dram_parameter("v_bias", [128, HD], f32, isOutput=False).ap()
    fc_w = nc.declare_dram_parameter("fc_w", [128, (HD // 128) * E], f16, isOutput=False).ap()
    out = nc.declare_dram_parameter("out", [T, E], f16, isOutput=True).ap()

    with tile.TileContext(nc) as tc, ExitStack() as ctx:
        pool_const = ctx.enter_context(tc.tile_pool(name="const", bufs=1))
        pool_kqv = ctx.enter_context(tc.tile_pool(name="kqv", bufs=1))
        pool_w = ctx.enter_context(tc.tile_pool(name="w", bufs=2))
        pool_xt = ctx.enter_context(tc.tile_pool(name="xt", bufs=1))
        pool_ex = ctx.enter_context(tc.tile_pool(name="ex", bufs=4))
        pool_outT = ctx.enter_context(tc.tile_pool(name="outT", bufs=1))
        pool_fc = ctx.enter_context(tc.tile_pool(name="fc", bufs=1))
        pool_ot = ctx.enter_context(tc.tile_pool(name="ot", bufs=4))
        ps_mm = ctx.enter_context(tc.tile_pool(name="psmm", bufs=2, space="PSUM"))
        ps_e = ctx.enter_context(tc.tile_pool(name="pse", bufs=1, space="PSUM"))
        ps_o = ctx.enter_context(tc.tile_pool(name="pso", bufs=1, space="PSUM"))

        # ---- persistent tiles ----
        vb_sb = pool_const.tile([128, HD], f32)
        qb_sb = pool_const.tile([128, NG * GC, 1], f32)
        fcw_sb = pool_fc.tile([128, HD // 128, E], f16)

        xts = [pool_xt.tile([128, EC, 512], f16, name=f"xt{tb}", tag=f"xt{tb}")
               for tb in range(NTB)]
        wks = [pool_w.tile([128, EC, 256], f16, name=f"wk{g}", tag="wk")
               for g in range(NG)]
        wvs = [pool_w.tile([128, EC, 256], f16, name=f"wv{g}", tag="wv")
               for g in range(NG)]
        wqs = [pool_w.tile([128, EC, 256], f16, name=f"wq{g}", tag="wq")
               for g in range(NG)]

        # ---- startup DMAs: interleave across the two HWDGE queues so the
        # first kproj chain (wk + xt0) lands as early as possible; everything
        # else (incl. group-1 weights and fc weights) streams in behind.
        def src_w(g, m):
            return wqkv[g, m].rearrange("p (c n) -> p c n", c=EC)

        def src_xt(tb):
            return xt4[tb].rearrange("p (c n) -> p c n", c=EC)

        # first working set (wk0 + xt0) in ec-pair chunks, interleaved across
        # the two queues, so the first kproj chain starts ~1us after the
        # first pair lands instead of waiting for whole tiles
        wk_src = src_w(0, 1)
        xt0_src = src_xt(0)
        for h in range(4):
            sl = slice(2 * h, 2 * h + 2)
            nc.sync.dma_start(out=wks[0][:, sl, :], in_=wk_src[:, sl, :])
            nc.scalar.dma_start(out=xts[0][:, sl, :], in_=xt0_src[:, sl, :])
        nc.sync.dma_start(out=wvs[0], in_=src_w(0, 2))
        nc.scalar.dma_start(out=xts[1], in_=src_xt(1))
        nc.sync.dma_start(out=vb_sb, in_=v_bias)
        nc.sync.dma_start(out=qb_sb, in_=q_bias)
        nc.sync.dma_start(out=wqs[0], in_=src_w(0, 0))

        # denominator-broadcast mask: psR = dmask.T @ dsb maps dsb row 64
        # (even-head denom) onto rows 0:64 and dsb row 0 (odd) onto 64:128
        dmask = pool_const.tile([128, 128], f16)
        nc.vector.memset(dmask, 0.0)
        nc.vector.memset(dmask[64:65, 0:64], 1.0)
        nc.vector.memset(dmask[0:1, 64:128], 1.0)

        # warm the exp table (the implicit ACT_TABLE_LOAD precedes this on
        # the scalar queue, after the xt0/xt1 triggers above)
        dummy = pool_const.tile([128, 1], f16)
        nc.scalar.activation(dummy, dmask[:, 0:1], AF.Exp, scale=0.125)

        nc.scalar.dma_start(out=xts[2], in_=src_xt(2))
        nc.scalar.dma_start(out=xts[3], in_=src_xt(3))
        nc.sync.dma_start(out=wks[1], in_=src_w(1, 1))
        nc.sync.dma_start(out=wvs[1], in_=src_w(1, 2))
        nc.sync.dma_start(out=wqs[1], in_=src_w(1, 0))
        nc.sync.dma_start(out=fcw_sb,
                          in_=fc_w.rearrange("p (c n) -> p c n", c=HD // 128))

        dsbs = []
        for i in range(2):
            d = pool_const.tile([128, 512], f16, name=f"dsb{i}")
            nc.vector.memset(d, 0.0)
            dsbs.append(d)

        # double-buffered k/q/v group tiles. Only va needs its ones/zero
        # strips initialized; the kT/qT off-halves are never read (QK
        # contracts over just the 64 data rows of each slab).
        kTs, qTs, vas = [], [], []
        for i in range(2):
            kT = pool_kqv.tile([128, HPG, T], f16, name=f"kT{i}", tag=f"kT{i}")
            qT = pool_kqv.tile([128, HPG, T], f16, name=f"qT{i}", tag=f"qT{i}")
            va = pool_kqv.tile([128, KC, GC, 192], f16, name=f"va{i}", tag=f"va{i}")
            nc.gpsimd.memset(va[:, :, :, 64:65], 1.0)
            nc.gpsimd.memset(va[:, :, :, 65:128], 0.0)
            kTs.append(kT)
            qTs.append(qT)
            vas.append(va)

        outT = pool_outT.tile([128, HD // 128, T], f16)

        # ---- phase A: projections for group g, as chunk closures.
        # Returns (pre, deferred, held):
        #   pre      — needed by b_iter(g, 0, 0): k-proj pair 0, all v-proj,
        #              q-proj window 0 pair 0
        #   deferred — needed by b_iter(g, 0, 1): k-proj pair 1, q window 0
        #              pair 1 (fed during the first b_iter)
        #   held     — q-projs for windows 1-3 (fed ahead of their window)
        def a_chunks(g):
            gi = g % 2
            kT, qT, va = kTs[gi], qTs[gi], vas[gi]
            wq, wk, wv = wqs[g], wks[g], wvs[g]

            def mk_kproj(tb, ch):
                def f():
                    xt_tb = xts[tb]
                    psA = ps_mm.tile([128, 512], f32, tag="mm")
                    for ec in range(EC):
                        nc.tensor.matmul(
                            psA, wk[:, ec, ch * 128:(ch + 1) * 128], xt_tb[:, ec, :],
                            start=(ec == 0), stop=(ec == EC - 1))
                    nc.vector.tensor_copy(
                        kT[0:64, 2 * ch, tb * 512:(tb + 1) * 512], psA[0:64, :])
                    nc.vector.tensor_copy(
                        kT[64:128, 2 * ch + 1, tb * 512:(tb + 1) * 512], psA[64:128, :])
                return f

            def mk_qproj(tb, ch):
                def f():
                    xt_tb = xts[tb]
                    P = g * GC + ch
                    psQ = ps_mm.tile([128, 512], f32, tag="mm")
                    for ec in range(EC):
                        nc.tensor.matmul(
                            psQ, wq[:, ec, ch * 128:(ch + 1) * 128], xt_tb[:, ec, :],
                            start=(ec == 0), stop=(ec == EC - 1))
                    nc.vector.tensor_scalar(
                        qT[0:64, 2 * ch, tb * 512:(tb + 1) * 512],
                        psQ[0:64, :], qb_sb[0:64, P, :], None, OP.add)
                    nc.vector.tensor_scalar(
                        qT[64:128, 2 * ch + 1, tb * 512:(tb + 1) * 512],
                        psQ[64:128, :], qb_sb[64:128, P, :], None, OP.add)
                return f

            def mk_vproj(tb, tc_):
                def f():
                    xt_tb = xts[tb]
                    psV = ps_mm.tile([128, 256], f32, tag="mm")
                    for ec in range(EC):
                        nc.tensor.matmul(
                            psV, xt_tb[:, ec, tc_ * 128:(tc_ + 1) * 128], wv[:, ec, :],
                            start=(ec == 0), stop=(ec == EC - 1))
                    psv4 = psV.rearrange("p (pr h d) -> p pr h d", pr=GC, h=2)
                    vb4 = vb_sb[:, g * 256:(g + 1) * 256].rearrange(
                        "p (pr h d) -> p pr h d", pr=GC, h=2)
                    K0 = tb * 4 + tc_
                    nc.vector.tensor_add(
                        va[:, K0, :, 0:64], psv4[:, :, 0, :], vb4[:, :, 0, :])
                    nc.vector.tensor_add(
                        va[:, K0, :, 128:192], psv4[:, :, 1, :], vb4[:, :, 1, :])
                return f

            pre, deferred, held = [], [], []
            for tb in range(NTB):
                pre.append(mk_kproj(tb, 0))
                for tc_ in range(4):
                    pre.append(mk_vproj(tb, tc_))
                deferred.append(mk_kproj(tb, 1))
            pre.append(mk_qproj(0, 0))
            deferred.append(mk_qproj(0, 1))
            for tb in range(1, NTB):
                held.append(mk_qproj(tb, 0))
                held.append(mk_qproj(tb, 1))
            return pre, deferred, held

        # ---- phase C: fc for one 128-query block (both 512-col halves).
        # hcs selects the contraction slices (head-pair blocks of fc rows).
        def fc_mm(qc, ob, hcs, psC, start):
            for i, hc in enumerate(hcs):
                nc.tensor.matmul(
                    psC, outT[:, hc, qc * 128:(qc + 1) * 128],
                    fcw_sb[:, hc, ob * 512:(ob + 1) * 512],
                    start=(start and i == 0), stop=(i == len(hcs) - 1))

        # fc feed granularity is a half-block (one 512-col half of one
        # 128-query block): ~2k PE cycles per feed item. The output DMA goes
        # out with the second half.
        ots = {}

        def fc_half(qc, ob):
            if ob == 0:
                ots[qc] = pool_ot.tile([128, E], f16, name=f"ot{qc}", tag="ot")
            ot = ots[qc]
            psC = ps_mm.tile([128, 512], f32, tag="mm")
            fc_mm(qc, ob, (0, 1, 2, 3), psC, True)
            nc.vector.tensor_copy(ot[:, ob * 512:(ob + 1) * 512], psC)
            if ob == 1:
                nc.sync.dma_start(out=out[qc * 128:(qc + 1) * 128, :], in_=ot)

        # split fc for the last window: partA (group-0 head pairs, ready
        # right after B(0)) runs early in B(1); partB finishes after the
        # last attention iteration with only 2 matmuls per half left.
        accs = {}

        def fc_partA(qc, ob):
            if ob == 0:
                accs[qc] = pool_ot.tile([128, E], f16, name=f"acc{qc}",
                                        tag=f"acc{qc % 4}", bufs=1)
            acc = accs[qc]
            psC = ps_mm.tile([128, 512], f32, tag="mm")
            fc_mm(qc, ob, (0, 1), psC, True)
            nc.vector.tensor_copy(acc[:, ob * 512:(ob + 1) * 512], psC)

        def fc_partB(qc, ob):
            acc = accs[qc]
            if ob == 0:
                ots[qc] = pool_ot.tile([128, E], f16, name=f"otb{qc}", tag="ot")
            ot = ots[qc]
            psC = ps_mm.tile([128, 512], f32, tag="mm")
            fc_mm(qc, ob, (2, 3), psC, True)
            osl = slice(ob * 512, (ob + 1) * 512)
            if qc < 14:
                # DVE path: add straight out of PSUM
                nc.vector.tensor_add(ot[:, osl], psC, acc[:, osl])
            else:
                # ACT+gpsimd path (both idle at the tail): ACT copies PSUM ->
                # SBUF, gpsimd (SBUF-only) adds. Halves the tail drain chain.
                tmp = pool_ot.tile([128, 512], f16, name=f"tmpB{qc}_{ob}",
                                   tag=f"tmpB{ob}", bufs=1)
                nc.scalar.copy(tmp, psC)
                nc.gpsimd.tensor_add(ot[:, osl], tmp, acc[:, osl])
            if ob == 1:
                nc.sync.dma_start(out=out[qc * 128:(qc + 1) * 128, :], in_=ot)

        # ---- phase B: attention for (g, qb-window, head pair p) ----
        def b_iter(g, qb, p, feeder, it, feed_ks):
            gi = g % 2
            kT, qT, va = kTs[gi], qTs[gi], vas[gi]
            psO_e = ps_o.tile([128, 512], f32, tag="po_e")
            psO_o = ps_o.tile([128, 512], f32, tag="po_o")

            def av(K, ex):
                blk = va[:, K, p, :]
                # even head: 65 stationary cols (64 dims + ones) suffice —
                # psO_e rows 65:127 are never read
                nc.tensor.matmul(
                    psO_e[0:65, :], blk[:, 0:65], ex[:, 0, :],
                    start=(K == 0), stop=(K == KC - 1))
                nc.tensor.matmul(
                    psO_o, blk[:, 64:192], ex[:, 1, :],
                    start=(K == 0), stop=(K == KC - 1))

            # software pipeline: AV of chunk K-1 is emitted after QK/ACT of
            # chunk K, so the in-order PE queue never stalls on ACT(K)
            prev = None
            for K in range(KC):
                psE = ps_e.tile([128, 2, 512], f32, tag=f"pe{K % 2}")
                for hf in range(2):
                    j = p * 2 + hf
                    # contract over just the 64 data rows of the slab
                    # (K=64): halves the stationary load vs the padded K=128
                    lo = 0 if j % 2 == 0 else 64
                    nc.tensor.matmul(
                        psE[:, hf, :],
                        kT[lo:lo + 64, j, K * 128:(K + 1) * 128],
                        qT[lo:lo + 64, j, qb * 512:(qb + 1) * 512],
                        start=True, stop=True)
                ex = pool_ex.tile([128, 2, 512], f16, tag="ex")
                nc.scalar.activation(ex, psE, AF.Exp, scale=SCALE)
                if prev is not None:
                    av(*prev)
                if K in feed_ks:
                    feeder()  # fill remaining slack with proj/fc matmuls
                prev = (K, ex)
            av(*prev)
            dsb = dsbs[it % 2]
            nc.vector.tensor_copy(dsb[64:65, :], psO_e[64:65, :])
            nc.vector.tensor_copy(dsb[0:1, :], psO_o[0:1, :])
            psR = ps_mm.tile([128, 512], f32, tag="mm")
            # rows 1..63 of dmask/dsb are all-zero: K=65 covers both
            # nonzero rows (0 and 64) exactly
            nc.tensor.matmul(psR, dmask[0:65, :], dsb[0:65, :],
                             start=True, stop=True)
            recipb = pool_ex.tile([128, 512], f32, tag="recipb")
            nc.vector.reciprocal_approx_fast(out=recipb, in_=psR)
            P = g * GC + p
            qs = slice(qb * 512, (qb + 1) * 512)
            nc.vector.tensor_tensor(
                outT[0:64, P, qs], psO_e[0:64, :], recipb[0:64, :], OP.mult)
            nc.vector.tensor_tensor(
                outT[64:128, P, qs], psO_o[64:128, :], recipb[64:128, :], OP.mult)

        # ---- main schedule ----
        # Only the chunks b_iter(0,0,0) needs run before B(0); everything
        # else is fill work fed into attention iterations, so the PE never
        # idles while the ACT engine streams exps.
        a0_pre, a0_def, a0_held = a_chunks(0)
        a1_pre, a1_def, a1_held = a_chunks(1)
        for c in a0_pre:
            c()

        # B(0) feed queue: pair-1 projections of group 0 first (q(0,1)
        # early so iter 1's QK unblocks), then group-0 held q-projs
        # interleaved with group-1 kv chunks (held q-proj (tb,*) must land
        # before window tb's iterations).
        a1_kv = a1_pre[:-1]      # 20 chunks: k-pair0 + v
        feed_items = [a0_def[0], a0_def[4], a0_def[1], a0_def[2], a0_def[3]]
        feed_items += a0_held[0:2] + a1_kv[0:7]
        feed_items += a0_held[2:4] + a1_kv[7:14]
        feed_items += a0_held[4:6] + a1_kv[14:20]
        feed_items += a1_def + [a1_pre[-1]]   # k-pair1, q(0,1), q(0,0)
        state = {"i": 0}

        def feeder():
            if state["i"] < len(feed_items):
                feed_items[state["i"]]()
                state["i"] += 1

        it = 0
        for qb in range(NQB):
            for p in range(GC):
                b_iter(0, qb, p, feeder, it, feed_ks=(1, 4, 7, 10, 13))
                it += 1
        while state["i"] < len(feed_items):
            feeder()

        # B(1) feed queue: held q-projs ahead of their windows, window-3 fc
        # partA (group-0 contraction half) early, fc halves as windows
        # complete, partB after the last iteration.
        feed_items.extend(a1_held[0:2])
        for qc in (12, 13):
            feed_items.append(lambda qc=qc: fc_partA(qc, 0))
            feed_items.append(lambda qc=qc: fc_partA(qc, 1))
        feed_items.extend(a1_held[2:4])
        for qc in (14, 15):
            feed_items.append(lambda qc=qc: fc_partA(qc, 0))
            feed_items.append(lambda qc=qc: fc_partA(qc, 1))
        feed_items.extend(a1_held[4:6])
        for qb in range(NQB):
            for p in range(GC):
                b_iter(1, qb, p, feeder, it, feed_ks=(1, 4, 7, 10, 13))
                it += 1
            if qb < 3:
                for qc in range(qb * 4, (qb + 1) * 4):
                    for ob in range(2):
                        feed_items.append(lambda qc=qc, ob=ob: fc_half(qc, ob))
        while state["i"] < len(feed_items):
            feeder()
        for qc in range(12, 16):
            fc_partB(qc, 0)
            fc_partB(qc, 1)

    nc.compile()
    return nc


def _get_nc():
    if "nc" not in _CACHE:
        _CACHE["nc"] = _build()
    return _CACHE["nc"]


def _in_maps(x, qkv_w, qkv_b, fc_w, fc_b):
    f16 = np.float16
    x = np.asarray(x, dtype=np.float32)
    qkv_w = np.asarray(qkv_w, dtype=np.float32)
    qkv_b = np.asarray(qkv_b, dtype=np.float32)
    fc_w = np.asarray(fc_w, dtype=np.float32)

    maps = []
    for c in range(N_CORES):
        b, s = c // 2, c % 2
        # xt4[tb, p, ec*512 + t] = x[b, tb*512 + t, ec*128 + p]
        xt4 = np.ascontiguousarray(
            x[b].astype(f16).reshape(NTB, 512, EC, 128).transpose(0, 3, 2, 1)
            .reshape(NTB, 128, EC * 512))
        # wqkv[g, m, p, ec*256 + n] = W_m[ec*128 + p, s*512 + g*256 + n]
        wg = np.empty((NG, 3, 128, EC * 256), dtype=f16)
        for m in range(3):
            sub = qkv_w[:, m * E + s * HD:m * E + (s + 1) * HD]  # [E, 512]
            for g in range(NG):
                blk = sub[:, g * 256:(g + 1) * 256]              # [E, 256]
                wg[g, m] = (blk.reshape(EC, 128, 256).transpose(1, 0, 2)
                            .reshape(128, EC * 256).astype(f16))
        qb = np.ascontiguousarray(
            qkv_b[s * HD:(s + 1) * HD].reshape(NG * GC, 128).T.reshape(128, NG * GC, 1),
            dtype=np.float32)
        vb = np.ascontiguousarray(
            np.broadcast_to(qkv_b[2 * E + s * HD:2 * E + (s + 1) * HD], (128, HD)),
            dtype=np.float32)
        # fc_ws[p, hc*1024 + n] = fc_w[s*512 + hc*128 + p, n]
        fcs = np.ascontiguousarray(
            fc_w[s * HD:(s + 1) * HD, :].reshape(HD // 128, 128, E)
            .transpose(1, 0, 2).reshape(128, (HD // 128) * E).astype(f16))
        maps.append({"xt4": xt4, "wqkv": wg, "q_bias": qb, "v_bias": vb,
                     "fc_w": fcs})
    return maps


def run(x, qkv_w, qkv_b, fc_w, fc_b, trace=False):
    from concourse.bass_utils import run_bass_kernel_spmd

    nc = _get_nc()
    maps = _in_maps(x, qkv_w, qkv_b, fc_w, fc_b)
    res = run_bass_kernel_spmd(nc, maps, list(range(N_CORES)), trace=trace)
    B = np.asarray(x).shape[0]
    fc_b = np.asarray(fc_b, dtype=np.float32)
    full = np.empty((B, T, E), dtype=np.float32)
    for b in range(B):
        full[b] = (res.results[2 * b]["out"].astype(np.float32)
                   + res.results[2 * b + 1]["out"].astype(np.float32) + fc_b)
    return full, res


def kernel(x, qkv_w, qkv_b, fc_w, fc_b):
    full, _ = run(x, qkv_w, qkv_b, fc_w, fc_b, trace=False)
    return full


# revision 37
# speedup vs baseline: 1.0263x; 1.0107x over previous
"""Multi-head self-attention (B=4, T=2048, E=1024, H=16, Dh=64) on 8 trn2 cores.

Sharding (tensor-parallel over heads + data-parallel over batch, per the
problem's sharding hint): core c handles batch b=c//2 and head-half s=c%2
(8 of 16 heads), with ALL 2048 queries of its batch. Each core computes
q/k/v projections for its 512 head-dims, attention for its 8 heads, and a
PARTIAL output projection (contracting only its 512 rows of fc_w). The two
partials per batch are summed on the host (the TP all-reduce), plus fc_b.

All matmul operands are fp16 (fp32 PSUM accumulation): fp16 streams 1
col/cycle through the PE vs 0.5 for fp32. Softmax denominators come free
from a ones-column in the packed V block; reciprocals are taken on the two
denominator rows BEFORE the mask-matmul broadcast. exp() runs as [128,2048]
ACTIVATEs (4 PSUM banks) to amortize the ~350cyc fixed cost.

Schedule: xt tiles are DMA'd ONCE and stay resident for both head groups;
all weight DMAs (both groups + fc) are prefetched at kernel start, split
across the two HWDGE queues (sync + scalar engines) so the first
projection chain starts as early as possible. Phase A (projections) of
group 1 feeds into phase B (attention) of group 0, and fc matmuls feed
into phase B of group 1, so the PE queue always has fill work during exp
stalls. Six group-1 q-projection chunks are held back to feed the first
B(1) iterations, and window 3's fc is split into two half-contractions so
the drain after the last attention iteration is minimal. Output is stored
fp16 (partials; exact sum happens on host in fp32).

Per-core math (heads processed in NG=2 groups of 4; head pair p packs its
even head at partitions 0:64 and odd head at 64:128):
  xt        [E, T]            (input, host pre-transposed/tiled, fp16)
  qT_j      [64, 2048] = Wq_j.T @ xt + bq_j      (slab-packed)
  kT_j      [64, 2048] = Wk_j.T @ xt             (k-bias is softmax-invariant)
  v_j       [2048, 64] = xt.T @ Wv_j + bv_j      (+ ones column for denom)
  e         [k,q] blocks via K=64 matmul over the slabs
  p = exp(e/8);  psO = [v | ones].T @ p  -> AV rows + denominator row
  outT      [64, q] per head = AV rows * recip(denom) (broadcast via dmask)
  out_part  [2048, 1024] = outT.T @ fc_w[s*512:(s+1)*512, :]   (partial)
"""

import numpy as np

E = 1024
T = 2048          # tokens per batch (= queries = keys per core)
H = 16
DH = 64
EC = E // 128     # 8 e-chunks
NG = 2            # head groups per core
HPG = 4           # heads per group
GC = HPG // 2     # head pairs per group = 2
NH = NG * HPG     # heads per core = 8
HD = NH * DH      # head-dims per core = 512
N_CORES = 8
SCALE = DH ** -0.5
KC = T // 128     # 16 key chunks
NTB = T // 512    # 4 token blocks
NQB = T // 512    # 4 query windows

_CACHE = {}


def _build():
    import concourse.bass as bass
    import concourse.mybir as mybir
    import concourse.tile as tile
    from concourse import bacc
    from contextlib import ExitStack

    f32 = mybir.dt.float32
    f16 = mybir.dt.float16
    AF = mybir.ActivationFunctionType
    OP = mybir.AluOpType

    nc = bacc.Bacc("TRN2", target_bir_lowering=False, debug=False)

    xt4 = nc.declare_dram_parameter("xt4", [NTB, 128, EC * 512], f16, isOutput=False).ap()
    wqkv = nc.declare_dram_parameter("wqkv", [NG, 3, 128, EC * 256], f16, isOutput=False).ap()
    q_bias = nc.declare_dram_parameter("q_bias", [128, NG * GC, 1], f32, isOutput=False).ap()
    v_bias = nc.declare_dram_parameter("v_bias", [128, HD], f32, isOutput=False).ap()
    fc_w = nc.declare_dram_parameter("fc_w", [128, (HD // 128) * E], f16, isOutput=False).ap()
    out = nc.declare_dram_parameter("out", [T, E], f16, isOutput=True).ap()

    with tile.TileContext(nc) as tc, ExitStack() as ctx:
        pool_const = ctx.enter_context(tc.tile_pool(name="const", bufs=1))
        pool_kqv = ctx.enter_context(tc.tile_pool(name="kqv", bufs=1))
        pool_w = ctx.enter_context(tc.tile_pool(name="w", bufs=2))
        pool_xt = ctx.enter_context(tc.tile_pool(name="xt", bufs=1))
        pool_ex = ctx.enter_context(tc.tile_pool(name="ex", bufs=4))
        pool_outT = ctx.enter_context(tc.tile_pool(name="outT", bufs=1))
        pool_fc = ctx.enter_context(tc.tile_pool(name="fc", bufs=1))
        pool_ot = ctx.enter_context(tc.tile_pool(name="ot", bufs=4))
        ps_mm = ctx.enter_context(tc.tile_pool(name="psmm", bufs=2, space="PSUM"))
        ps_e = ctx.enter_context(tc.tile_pool(name="pse", bufs=1, space="PSUM"))
        ps_o = ctx.enter_context(tc.tile_pool(name="pso", bufs=1, space="PSUM"))

        # ---- persistent tiles ----
        vb_sb = pool_const.tile([128, HD], f32)
        qb_sb = pool_const.tile([128, NG * GC, 1], f32)
        fcw_sb = pool_fc.tile([128, HD // 128, E], f16)

        xts = [pool_xt.tile([128, EC, 512], f16, name=f"xt{tb}", tag=f"xt{tb}")
               for tb in range(NTB)]
        wks = [pool_w.tile([128, EC, 256], f16, name=f"wk{g}", tag="wk")
               for g in range(NG)]
        wvs = [pool_w.tile([128, EC, 256], f16, name=f"wv{g}", tag="wv")
               for g in range(NG)]
        wqs = [pool_w.tile([128, EC, 256], f16, name=f"wq{g}", tag="wq")
               for g in range(NG)]

        # ---- startup DMAs: interleave across the two HWDGE queues so the
        # first kproj chain (wk + xt0) lands as early as possible; everything
        # else (incl. group-1 weights and fc weights) streams in behind.
        def src_w(g, m):
            return wqkv[g, m].rearrange("p (c n) -> p c n", c=EC)

        def src_xt(tb):
            return xt4[tb].rearrange("p (c n) -> p c n", c=EC)

        # first working set (wk0 + xt0) in ec-pair chunks, interleaved across
        # the two queues, so the first kproj chain starts ~1us after the
        # first pair lands instead of waiting for whole tiles
        wk_src = src_w(0, 1)
        xt0_src = src_xt(0)
        for h in range(4):
            sl = slice(2 * h, 2 * h + 2)
            nc.sync.dma_start(out=wks[0][:, sl, :], in_=wk_src[:, sl, :])
            nc.scalar.dma_start(out=xts[0][:, sl, :], in_=xt0_src[:, sl, :])
        nc.sync.dma_start(out=wvs[0], in_=src_w(0, 2))
        nc.scalar.dma_start(out=xts[1], in_=src_xt(1))
        nc.sync.dma_start(out=vb_sb, in_=v_bias)
        nc.sync.dma_start(out=qb_sb, in_=q_bias)
        nc.sync.dma_start(out=wqs[0], in_=src_w(0, 0))

        # denominator-broadcast mask: psR = dmask.T @ dsb maps dsb row 64
        # (even-head denom) onto rows 0:64 and dsb row 0 (odd) onto 64:128
        dmask = pool_const.tile([128, 128], f16)
        nc.vector.memset(dmask, 0.0)
        nc.vector.memset(dmask[64:65, 0:64], 1.0)
        nc.vector.memset(dmask[0:1, 64:128], 1.0)

        # warm the exp table (the implicit ACT_TABLE_LOAD precedes this on
        # the scalar queue, after the xt0/xt1 triggers above)
        dummy = pool_const.tile([128, 1], f16)
        nc.scalar.activation(dummy, dmask[:, 0:1], AF.Exp, scale=0.125)

        nc.scalar.dma_start(out=xts[2], in_=src_xt(2))
        nc.scalar.dma_start(out=xts[3], in_=src_xt(3))
        nc.sync.dma_start(out=wks[1], in_=src_w(1, 1))
        nc.sync.dma_start(out=wvs[1], in_=src_w(1, 2))
        nc.sync.dma_start(out=wqs[1], in_=src_w(1, 0))
        nc.sync.dma_start(out=fcw_sb,
                          in_=fc_w.rearrange("p (c n) -> p c n", c=HD // 128))

        dsbs = []
        for i in range(2):
            d = pool_const.tile([128, 512], f16, name=f"dsb{i}")
            nc.vector.memset(d, 0.0)
            dsbs.append(d)

        # double-buffered k/q/v group tiles. Only va needs its ones/zero
        # strips initialized; the kT/qT off-halves are never read (QK
        # contracts over just the 64 data rows of each slab).
        kTs, qTs, vas = [], [], []
        for i in range(2):
            kT = pool_kqv.tile([128, HPG, T], f16, name=f"kT{i}", tag=f"kT{i}")
            qT = pool_kqv.tile([128, HPG, T], f16, name=f"qT{i}", tag=f"qT{i}")
            va = pool_kqv.tile([128, KC, GC, 192], f16, name=f"va{i}", tag=f"va{i}")
            nc.gpsimd.memset(va[:, :, :, 64:65], 1.0)
            nc.gpsimd.memset(va[:, :, :, 65:128], 0.0)
            kTs.append(kT)
            qTs.append(qT)
            vas.append(va)

        outT = pool_outT.tile([128, HD // 128, T], f16)

        # ---- phase A: projections for group g, as chunk closures.
        # Returns (pre, deferred, held):
        #   pre      — needed by b_iter(g, 0, 0): k-proj pair 0, all v-proj,
        #              q-proj window 0 pair 0
        #   deferred — needed by b_iter(g, 0, 1): k-proj pair 1, q window 0
        #              pair 1 (fed during the first b_iter)
        #   held     — q-projs for windows 1-3 (fed ahead of their window)
        def a_chunks(g):
            gi = g % 2
            kT, qT, va = kTs[gi], qTs[gi], vas[gi]
            wq, wk, wv = wqs[g], wks[g], wvs[g]

            def mk_kproj(tb, ch):
                def f():
                    xt_tb = xts[tb]
                    psA = ps_mm.tile([128, 512], f32, tag="mm")
                    for ec in range(EC):
                        nc.tensor.matmul(
                            psA, wk[:, ec, ch * 128:(ch + 1) * 128], xt_tb[:, ec, :],
                            start=(ec == 0), stop=(ec == EC - 1))
                    nc.vector.tensor_copy(
                        kT[0:64, 2 * ch, tb * 512:(tb + 1) * 512], psA[0:64, :])
                    nc.vector.tensor_copy(
                        kT[64:128, 2 * ch + 1, tb * 512:(tb + 1) * 512], psA[64:128, :])
                return f

            def mk_qproj(tb, ch):
                def f():
                    xt_tb = xts[tb]
                    P = g * GC + ch
                    psQ = ps_mm.tile([128, 512], f32, tag="mm")
                    for ec in range(EC):
                        nc.tensor.matmul(
                            psQ, wq[:, ec, ch * 128:(ch + 1) * 128], xt_tb[:, ec, :],
                            start=(ec == 0), stop=(ec == EC - 1))
                    nc.vector.tensor_scalar(
                        qT[0:64, 2 * ch, tb * 512:(tb + 1) * 512],
                        psQ[0:64, :], qb_sb[0:64, P, :], None, OP.add)
                    nc.vector.tensor_scalar(
                        qT[64:128, 2 * ch + 1, tb * 512:(tb + 1) * 512],
                        psQ[64:128, :], qb_sb[64:128, P, :], None, OP.add)
                return f

            def mk_vproj(tb, tc_):
                def f():
                    xt_tb = xts[tb]
                    psV = ps_mm.tile([128, 256], f32, tag="mm")
                    for ec in range(EC):
                        nc.tensor.matmul(
                            psV, xt_tb[:, ec, tc_ * 128:(tc_ + 1) * 128], wv[:, ec, :],
                            start=(ec == 0), stop=(ec == EC - 1))
                    psv4 = psV.rearrange("p (pr h d) -> p pr h d", pr=GC, h=2)
                    vb4 = vb_sb[:, g * 256:(g + 1) * 256].rearrange(
                        "p (pr h d) -> p pr h d", pr=GC, h=2)
                    K0 = tb * 4 + tc_
                    nc.vector.tensor_add(
                        va[:, K0, :, 0:64], psv4[:, :, 0, :], vb4[:, :, 0, :])
                    nc.vector.tensor_add(
                        va[:, K0, :, 128:192], psv4[:, :, 1, :], vb4[:, :, 1, :])
                return f

            pre, deferred, held = [], [], []
            for tb in range(NTB):
                pre.append(mk_kproj(tb, 0))
                for tc_ in range(4):
                    pre.append(mk_vproj(tb, tc_))
                deferred.append(mk_kproj(tb, 1))
            pre.append(mk_qproj(0, 0))
            deferred.append(mk_qproj(0, 1))
            for tb in range(1, NTB):
                held.append(mk_qproj(tb, 0))
                held.append(mk_qproj(tb, 1))
            return pre, deferred, held

        # ---- phase C: fc for one 128-query block (both 512-col halves).
        # hcs selects the contraction slices (head-pair blocks of fc rows).
        def fc_mm(qc, ob, hcs, psC, start):
            for i, hc in enumerate(hcs):
                nc.tensor.matmul(
                    psC, outT[:, hc, qc * 128:(qc + 1) * 128],
                    fcw_sb[:, hc, ob * 512:(ob + 1) * 512],
                    start=(start and i == 0), stop=(i == len(hcs) - 1))

        # fc feed granularity is a half-block (one 512-col half of one
        # 128-query block): ~2k PE cycles per feed item. The output DMA goes
        # out with the second half.
        ots = {}

        def fc_half(qc, ob):
            if ob == 0:
                ots[qc] = pool_ot.tile([128, E], f16, name=f"ot{qc}", tag="ot")
            ot = ots[qc]
            psC = ps_mm.tile([128, 512], f32, tag="mm")
            fc_mm(qc, ob, (0, 1, 2, 3), psC, True)
            nc.vector.tensor_copy(ot[:, ob * 512:(ob + 1) * 512], psC)
            if ob == 1:
                nc.sync.dma_start(out=out[qc * 128:(qc + 1) * 128, :], in_=ot)

        # split fc for the last window: partA (group-0 head pairs, ready
        # right after B(0)) runs early in B(1); partB finishes after the
        # last attention iteration with only 2 matmuls per half left.
        accs = {}

        def fc_partA(qc, ob):
            if ob == 0:
                accs[qc] = pool_ot.tile([128, E], f16, name=f"acc{qc}",
                                        tag=f"acc{qc % 4}", bufs=1)
            acc = accs[qc]
            psC = ps_mm.tile([128, 512], f32, tag="mm")
            fc_mm(qc, ob, (0, 1), psC, True)
            nc.vector.tensor_copy(acc[:, ob * 512:(ob + 1) * 512], psC)

        def fc_partB(qc, ob):
            acc = accs[qc]
            if ob == 0:
                ots[qc] = pool_ot.tile([128, E], f16, name=f"otb{qc}", tag="ot")
            ot = ots[qc]
            psC = ps_mm.tile([128, 512], f32, tag="mm")
            fc_mm(qc, ob, (2, 3), psC, True)
            osl = slice(ob * 512, (ob + 1) * 512)
            nc.vector.tensor_add(ot[:, osl], psC, acc[:, osl])
            if ob == 1:
                nc.sync.dma_start(out=out[qc * 128:(qc + 1) * 128, :], in_=ot)

        # ---- phase B: attention for (g, qb-window, head pair p) ----
        def b_iter(g, qb, p, feeder, it, feed_ks):
            gi = g % 2
            kT, qT, va = kTs[gi], qTs[gi], vas[gi]
            psO_e = ps_o.tile([128, 512], f32, tag="po_e")
            psO_o = ps_o.tile([128, 512], f32, tag="po_o")

            def av(K, ex):
                blk = va[:, K, p, :]
                # even head: 65 stationary cols (64 dims + ones) suffice —
                # psO_e rows 65:127 are never read
                nc.tensor.matmul(
                    psO_e[0:65, :], blk[:, 0:65], ex[:, 0, :],
                    start=(K == 0), stop=(K == KC - 1))
                nc.tensor.matmul(
                    psO_o, blk[:, 64:192], ex[:, 1, :],
                    start=(K == 0), stop=(K == KC - 1))

            # software pipeline: AV of chunk K-1 is emitted after QK/ACT of
            # chunk K, so the in-order PE queue never stalls on ACT(K)
            prev = None
            for K in range(KC):
                psE = ps_e.tile([128, 2, 512], f32, tag=f"pe{K % 2}")
                for hf in range(2):
                    j = p * 2 + hf
                    # contract over just the 64 data rows of the slab
                    # (K=64): halves the stationary load vs the padded K=128
                    lo = 0 if j % 2 == 0 else 64
                    nc.tensor.matmul(
                        psE[:, hf, :],
                        kT[lo:lo + 64, j, K * 128:(K + 1) * 128],
                        qT[lo:lo + 64, j, qb * 512:(qb + 1) * 512],
                        start=True, stop=True)
                ex = pool_ex.tile([128, 2, 512], f16, tag="ex")
                nc.scalar.activation(ex, psE, AF.Exp, scale=SCALE)
                if prev is not None:
                    av(*prev)
                if K in feed_ks:
                    feeder()  # fill remaining slack with proj/fc matmuls
                prev = (K, ex)
            av(*prev)
            dsb = dsbs[it % 2]
            nc.vector.tensor_copy(dsb[64:65, :], psO_e[64:65, :])
            nc.vector.tensor_copy(dsb[0:1, :], psO_o[0:1, :])
            psR = ps_mm.tile([128, 512], f32, tag="mm")
            # rows 1..63 of dmask/dsb are all-zero: K=65 covers both
            # nonzero rows (0 and 64) exactly
            nc.tensor.matmul(psR, dmask[0:65, :], dsb[0:65, :],
                             start=True, stop=True)
            recipb = pool_ex.tile([128, 512], f32, tag="recipb")
            nc.vector.reciprocal_approx_fast(out=recipb, in_=psR)
            P = g * GC + p
            qs = slice(qb * 512, (qb + 1) * 512)
            nc.vector.tensor_tensor(
                outT[0:64, P, qs], psO_e[0:64, :], recipb[0:64, :], OP.mult)
            nc.vector.tensor_tensor(
                outT[64:128, P, qs], psO_o[64:128, :], recipb[64:128, :], OP.mult)

        # ---- main schedule ----
        # Only the chunks b_iter(0,0,0) needs run before B(0); everything
        # else is fill work fed into attention iterations, so the PE never
        # idles while the ACT engine streams exps.
        a0_pre, a0_def, a0_held = a_chunks(0)
        a1_pre, a1_def, a1_held = a_chunks(1)
        for c in a0_pre:
            c()

        # B(0) feed queue: pair-1 projections of group 0 first (q(0,1)
        # early so iter 1's QK unblocks), then group-0 held q-projs
        # interleaved with group-1 kv chunks (held q-proj (tb,*) must land
        # before window tb's iterations).
        a1_kv = a1_pre[:-1]      # 20 chunks: k-pair0 + v
        feed_items = list(a0_def)
        feed_items += a0_held[0:2] + a1_kv[0:7]
        feed_items += a0_held[2:4] + a1_kv[7:14]
        feed_items += a0_held[4:6] + a1_kv[14:20]
        feed_items += a1_def + [a1_pre[-1]]   # k-pair1, q(0,1), q(0,0)
        state = {"i": 0}

        def feeder():
            if state["i"] < len(feed_items):
                feed_items[state["i"]]()
                state["i"] += 1

        it = 0
        for qb in range(NQB):
            for p in range(GC):
                b_iter(0, qb, p, feeder, it, feed_ks=(1, 4, 7, 10, 13))
                it += 1
        while state["i"] < len(feed_items):
            feeder()

        # B(1) feed queue: held q-projs ahead of their windows, window-3 fc
        # partA (group-0 contraction half) early, fc halves as windows
        # complete, partB after the last iteration.
        feed_items.extend(a1_held[0:2])
        for qc in (12, 13):
            feed_items.append(lambda qc=qc: fc_partA(qc, 0))
            feed_items.append(lambda qc=qc: fc_partA(qc, 1))
        feed_items.extend(a1_held[2:4])
        for qc in (14, 15):
            feed_items.append(lambda qc=qc: fc_partA(qc, 0))
            feed_items.append(lambda qc=qc: fc_partA(qc, 1))
        feed_items.extend(a1_held[4:6])
        for qb in range(NQB):
            for p in range(GC):
                b_iter(1, qb, p, feeder, it, feed_ks=(1, 4, 7, 10, 13))
                it += 1
            if qb < 3:
                for qc in range(qb * 4, (qb + 1) * 4):
                    for ob in range(2):
                        feed_items.append(lambda qc=qc, ob=ob: fc_half(qc, ob))
        while state["i"] < len(feed_items):
            feeder()
        for qc in range(12, 16):
            fc_partB(qc, 0)
            fc_partB(qc, 1)

    nc.compile()
    return nc


def _get_nc():
    if "nc" not in _CACHE:
        _CACHE["nc"] = _build()
    return _CACHE["nc"]


def _in_maps(x, qkv_w, qkv_b, fc_w, fc_b):
    f16 = np.float16
    x = np.asarray(x, dtype=np.float32)
    qkv_w = np.asarray(qkv_w, dtype=np.float32)
    qkv_b = np.asarray(qkv_b, dtype=np.float32)
    fc_w = np.asarray(fc_w, dtype=np.float32)

    maps = []
    for c in range(N_CORES):
        b, s = c // 2, c % 2
        # xt4[tb, p, ec*512 + t] = x[b, tb*512 + t, ec*128 + p]
        xt4 = np.ascontiguousarray(
            x[b].astype(f16).reshape(NTB, 512, EC, 128).transpose(0, 3, 2, 1)
            .reshape(NTB, 128, EC * 512))
        # wqkv[g, m, p, ec*256 + n] = W_m[ec*128 + p, s*512 + g*256 + n]
        wg = np.empty((NG, 3, 128, EC * 256), dtype=f16)
        for m in range(3):
            sub = qkv_w[:, m * E + s * HD:m * E + (s + 1) * HD]  # [E, 512]
            for g in range(NG):
                blk = sub[:, g * 256:(g + 1) * 256]              # [E, 256]
                wg[g, m] = (blk.reshape(EC, 128, 256).transpose(1, 0, 2)
                            .reshape(128, EC * 256).astype(f16))
        qb = np.ascontiguousarray(
            qkv_b[s * HD:(s + 1) * HD].reshape(NG * GC, 128).T.reshape(128, NG * GC, 1),
            dtype=np.float32)
        vb = np.ascontiguousarray(
            np.broadcast_to(qkv_b[2 * E + s * HD:2 * E + (s + 1) * HD], (128, HD)),
            dtype=np.float32)
        # fc_ws[p, hc*1024 + n] = fc_w[s*512 + hc*128 + p, n]
        fcs = np.ascontiguousarray(
            fc_w[s * HD:(s + 1) * HD, :].reshape(HD // 128, 128, E)
            .transpose(1, 0, 2).reshape(128, (HD // 128) * E).astype(f16))
        maps.append({"xt4": xt4, "wqkv": wg, "q_bias": qb, "v_bias": vb,
                     "fc_w": fcs})
    return maps


def run(x, qkv_w, qkv_b, fc_w, fc_b, trace=False):
    from concourse.bass_utils import run_bass_kernel_spmd

    nc = _get_nc()
    maps = _in_maps(x, qkv_w, qkv_b, fc_w, fc_b)
    res = run_bass_kernel_spmd(nc, maps, list(range(N_CORES)), trace=trace)
    B = np.asarray(x).shape[0]
    fc_b = np.asarray(fc_b, dtype=np.float32)
    full = np.empty((B, T, E), dtype=np.float32)
    for b in range(B):
        full[b] = (res.results[2 * b]["out"].astype(np.float32)
                   + res.results[2 * b + 1]["out"].astype(np.float32) + fc_b)
    return full, res


def kernel(x, qkv_w, qkv_b, fc_w, fc_b):
    full, _ = run(x, qkv_w, qkv_b, fc_w, fc_b, trace=False)
    return full
